# revision 2
# baseline (speedup 1.0000x reference)
"""Trainium2 Bass kernel for LocalDenseConv2D + BatchNorm + PReLU (v7: v6 +
  - x chunks split across both HW DGE queues so the x stream completes
    before the runtime CC barrier begins throttling DMA (~20us in),
  - last-pair drain restructured: Square-from-PSUM (S2) and a vector
    reduce-from-PSUM + 2048*bias fold (S) run in parallel; the ot drain
    Identity runs after, off the collective trigger path).

Sharding: out_l across 8 cores (8 rows each), all batches per core.

v3 changes vs v2:
  - x DRAM/SBUF layout [128, SLAB, 4, TP]: slab-row DMAs are contiguous
    2064B per partition (4KB-class packets, ~2x DMA bandwidth).
  - DMA split across the two HWDGE queues: x rows on sync (SP), weights +
    consts + half the gathers/stores on scalar (Activation).
  - Early dummy Sqrt activation loads the sqrt table set during the DMA
    preamble; Identity/Prelu are fillers in every set, so no table load
    sits on the post-collective critical path.
  - BN stats via sums: the PSUM-drain Identity produces sum((conv+bias))
    through accum_out; a vector tensor_tensor_reduce on the bf16 ot tile
    produces sum((conv+bias)^2). AllGather ships (S, S2) per (parity,
    channel); a single tensor_reduce merges the 16 groups. This also
    fixes v1/v2's stats-missing-bias bug.
  - ot is bf16 (halves SBUF + doubles DVE throughput on the apply).
  - Apply split: pairs 0-1 scalar Prelu, pairs 2-3 DVE 3-op prelu
    (max(t, alpha*t), valid for alpha in [0,1]).
"""
import sys
import numpy as np
import ml_dtypes

if '/opt/trn_rl_repo' not in sys.path:
    sys.path.insert(0, '/opt/trn_rl_repo')

import concourse.bass as bass
import concourse.bacc as bacc
import concourse.mybir as mybir
import concourse.tile as tile
from concourse.bass_utils import run_bass_kernel_spmd

F32 = mybir.dt.float32
BF16 = mybir.dt.bfloat16
AF = mybir.ActivationFunctionType
ALU = mybir.AluOpType
BFNP = ml_dtypes.bfloat16

B, IN_C, L, T = 8, 64, 64, 256
OUT_C, OUT_L = 64, 64
NCORES = 8
L_LOC = L // NCORES          # 8 out_l rows per core
NPAIR = L_LOC // 2           # 4 lp pairs (even/odd parity on PSUM halves)
SLAB = L_LOC + 2             # 10 x-rows incl. halo
TP = T + 2                   # padded t
EPS = 1e-5
N_GLOBAL = B * L * T         # 131072

_cache = {}


def _build():
    nc = bacc.Bacc("TRN2", target_bir_lowering=False, debug=False,
                   num_devices=NCORES)
    xr = nc.dram_tensor("xr", [128, SLAB, 4, TP], BF16, kind="ExternalInput")
    wr = nc.dram_tensor("wr", [128, 9, L_LOC, OUT_C], BF16, kind="ExternalInput")
    # combined consts: [0:NPAIR]=bias pairs, NPAIR=gamma, NPAIR+1=beta, NPAIR+2=alpha
    cr = nc.dram_tensor("cr", [128, NPAIR + 3], F32, kind="ExternalInput")
    yo = nc.dram_tensor("yo", [128, NPAIR, 2048], BF16, kind="ExternalOutput")

    cc_in = nc.dram_tensor("cc_in", [128, 2], F32)
    cc_out = nc.dram_tensor("cc_out", [NCORES * 128, 2], F32, addr_space="Shared")
    cc_din = nc.dram_tensor("cc_din", [128, 2], F32)
    cc_dout = nc.dram_tensor("cc_dout", [NCORES * 128, 2], F32, addr_space="Shared")

    with tile.TileContext(nc) as tc:
        with (
            tc.tile_pool(name="const", bufs=1) as cpool,
            tc.tile_pool(name="xp", bufs=1) as xpool,
            tc.tile_pool(name="op", bufs=1) as opool,
            tc.tile_pool(name="fp", bufs=3) as fpool,
            tc.tile_pool(name="ps", bufs=2, space="PSUM") as ppool,
        ):
            wt = cpool.tile([128, 9, L_LOC, OUT_C], BF16)
            ct = cpool.tile([128, NPAIR + 3], F32)
            xt = xpool.tile([128, SLAB, 4, TP], BF16)
            ot = opool.tile([128, NPAIR, 2048], BF16)
            acst = cpool.tile([128, 2, NPAIR], F32)
            epst = cpool.tile([128, 1], F32)
            dum = cpool.tile([128, 1], F32)

            # weights + consts on the scalar HWDGE queue, x on sync
            nc.scalar.dma_start(wt[:, 0:1], wr.ap()[:, 0:1])
            for s in range(0, SLAB, 2):
                if (s // 2) % 2 == 0:
                    nc.sync.dma_start(xt[:, s:s + 2], xr.ap()[:, s:s + 2])
                else:
                    nc.scalar.dma_start(xt[:, s:s + 2], xr.ap()[:, s:s + 2])
            nc.scalar.dma_start(ct[:], cr.ap())
            nc.scalar.dma_start(wt[:, 1:9], wr.ap()[:, 1:9])
            # preload the sqrt table set while DMAs stream
            nc.vector.memset(epst[:], EPS)
            nc.scalar.activation(dum[:], epst[:], AF.Sqrt, bias=epst[:])
            # dummy collective at launch: pre-pay CC stream setup
            dm2 = cpool.tile([128, 2], F32)
            nc.vector.memset(dm2[:], 1.0)
            nc.scalar.dma_start(cc_din.ap(), dm2[:])
            nc.gpsimd.collective_compute(
                "AllGather", ALU.bypass,
                replica_groups=[list(range(NCORES))],
                ins=[cc_din[:]], outs=[cc_dout[:]])

            sb3 = cpool.tile([128, 1], F32)
            rsum = cpool.tile([128, 1], F32)
            nc.vector.tensor_scalar(sb3[:], ct[:, NPAIR - 1:NPAIR], 2048.0, None,
                                    ALU.mult)

            # ---- conv: 4 concurrent 64x64 PE tiles (bh rows x parity cols)
            for pair in range(NPAIR):
                pt = ppool.tile([128, 2, 2, 512], F32, tag="p")
                for combo in range(9):
                    di, dj = combo // 3, combo % 3
                    first = combo == 0
                    last = combo == 8
                    for nt in range(2):
                        for bh in range(2):
                            for pi in range(2):
                                lp = 2 * pair + pi
                                nc.tensor.matmul(
                                    pt[64 * pi:64 * pi + 64, bh, nt, :],
                                    wt[64 * bh:64 * bh + 64, combo, lp, :],
                                    xt[64 * bh:64 * bh + 64,
                                       lp + di,
                                       2 * nt:2 * nt + 2,
                                       dj:dj + T],
                                    start=first, stop=last)
                sq = fpool.tile([128, 2048], BF16, tag="sq")
                if pair < NPAIR - 1:
                    # drain + sum((conv+bias)) via accum, then sumsq
                    nc.scalar.activation(
                        ot[:, pair, :],
                        pt[:].rearrange("p a b n -> p (a b n)"),
                        AF.Identity, bias=ct[:, pair:pair + 1],
                        accum_out=acst[:, 0, pair:pair + 1])
                    nc.scalar.activation(
                        sq[:], ot[:, pair, :], AF.Square,
                        accum_out=acst[:, 1, pair:pair + 1])
                else:
                    # last pair: trigger path needs only S and S2.
                    # scalar: S2 = sum((psum+bias)^2); vector: S via
                    # reduce(psum) + 2048*bias; Identity drain afterwards.
                    nc.scalar.activation(
                        sq[:], pt[:].rearrange("p a b n -> p (a b n)"),
                        AF.Square, bias=ct[:, pair:pair + 1],
                        accum_out=acst[:, 1, pair:pair + 1])
                    nc.vector.tensor_reduce(
                        rsum[:], pt[:].rearrange("p a b n -> p (a b n)"),
                        mybir.AxisListType.X, ALU.add)
                    nc.vector.tensor_tensor(
                        acst[:, 0, pair:pair + 1], rsum[:], sb3[:], ALU.add)
                    nc.scalar.activation(
                        ot[:, pair, :],
                        pt[:].rearrange("p a b n -> p (a b n)"),
                        AF.Identity, bias=ct[:, pair:pair + 1])

            # ---- local sums -> collective -> global scale/shift ----
            loc = cpool.tile([128, 2, 1], F32)
            nc.vector.tensor_reduce(loc[:], acst[:], mybir.AxisListType.X, ALU.add)
            nc.sync.dma_start(cc_in.ap(), loc[:, :, 0])
            nc.gpsimd.collective_compute(
                "AllGather", ALU.bypass,
                replica_groups=[list(range(NCORES))],
                ins=[cc_in[:]], outs=[cc_out[:]])
            gath = cpool.tile([128, 2, 2 * NCORES], F32)
            nc.sync.dma_start(
                gath[0:64],
                cc_out.ap().rearrange("(r h c) s -> c s (r h)", h=2, c=OUT_C))
            nc.scalar.dma_start(
                gath[64:128],
                cc_out.ap().rearrange("(r h c) s -> c s (r h)", h=2, c=OUT_C))

            gstat = cpool.tile([128, 2, 1], F32)
            nc.vector.tensor_reduce(gstat[:], gath[:], mybir.AxisListType.X, ALU.add)
            gsc = cpool.tile([128, 2], F32)
            nc.vector.tensor_scalar(gsc[:], gstat[:, :, 0], 1.0 / N_GLOBAL, None,
                                    ALU.mult)
            # var = E[x^2] - mean^2 ; sca = gamma/sqrt(var+eps)
            # shi = beta - mean*sca
            var = cpool.tile([128, 1], F32)
            std = cpool.tile([128, 1], F32)
            rstd = cpool.tile([128, 1], F32)
            sca = cpool.tile([128, 1], F32)
            shi = cpool.tile([128, 1], F32)
            nc.vector.tensor_tensor(var[:], gsc[:, 0:1], gsc[:, 0:1], ALU.mult)
            nc.vector.tensor_tensor(var[:], gsc[:, 1:2], var[:], ALU.subtract)
            nc.scalar.activation(std[:], var[:], AF.Sqrt, bias=epst[:])
            nc.vector.reciprocal(rstd[:], std[:])
            nc.vector.tensor_tensor(sca[:], ct[:, NPAIR:NPAIR + 1], rstd[:], ALU.mult)
            nc.vector.tensor_tensor(shi[:], gsc[:, 0:1], sca[:], ALU.mult)
            nc.vector.tensor_tensor(shi[:], ct[:, NPAIR + 1:NPAIR + 2], shi[:], ALU.subtract)

            # ---- fused BN-apply + PReLU + store (bf16) ----
            # pairs 0-1 on scalar (Prelu), pairs 2-3 on DVE (max(t, alpha*t))
            for pair in range(2):
                fo = fpool.tile([128, 2048], BF16, tag="fo")
                nc.scalar.activation(
                    fo[:], ot[:, pair, :],
                    AF.Prelu, bias=shi[:], scale=sca[:], alpha=ct[:, NPAIR + 2:NPAIR + 3])
                nc.sync.dma_start(yo.ap()[:, pair, :], fo[:])
            for pair in range(2, NPAIR):
                tv = fpool.tile([128, 2048], BF16, tag="tv")
                tu = fpool.tile([128, 2048], BF16, tag="tu")
                nc.vector.tensor_scalar(tv[:], ot[:, pair, :], sca[:], shi[:],
                                        ALU.mult, ALU.add)
                nc.vector.tensor_scalar(tu[:], tv[:], ct[:, NPAIR + 2:NPAIR + 3], None, ALU.mult)
                nc.vector.tensor_tensor(tu[:], tv[:], tu[:], ALU.max)
                nc.scalar.dma_start(yo.ap()[:, pair, :], tu[:])
    nc.compile()
    return nc


def _prep(x, weight, bias, gamma, beta, alpha):
    """Build per-core input maps (host-side shard + relayout, bf16)."""
    in_maps = []
    xpad = np.zeros((B, IN_C, L + 2, TP), np.float32)
    xpad[:, :, 1:L + 1, 1:T + 1] = x
    xpad = xpad.astype(BFNP)
    wbf = weight.astype(BFNP)
    g2 = np.concatenate([gamma, gamma]).reshape(128, 1).astype(np.float32)
    e2 = np.concatenate([beta, beta]).reshape(128, 1).astype(np.float32)
    a2 = np.full((128, 1), float(alpha[0]), np.float32)
    for r in range(NCORES):
        l0 = r * L_LOC
        # x: [bh*64+ci, slab_row, b4, 1+t]
        slab = xpad[:, :, l0:l0 + SLAB, :]          # (B, IN_C, SLAB, TP)
        xr = slab.reshape(2, 4, IN_C, SLAB, TP).transpose(0, 2, 3, 1, 4)
        xr = np.ascontiguousarray(xr.reshape(128, SLAB, 4, TP))
        # weight: [bh*64+ci, combo, lp, c] = weight[ci*9+combo, c, l0+lp]
        wl = wbf[:, :, l0:l0 + L_LOC]               # (576, 64, 8)
        wl = wl.reshape(IN_C, 9, OUT_C, L_LOC).transpose(0, 1, 3, 2)
        wr = np.ascontiguousarray(
            np.broadcast_to(wl[None], (2, IN_C, 9, L_LOC, OUT_C))
            .reshape(128, 9, L_LOC, OUT_C))
        # bias: [pi*64+c, pair] = bias[c, l0 + 2*pair + pi]
        bl = bias[:, l0:l0 + L_LOC].reshape(OUT_C, NPAIR, 2)
        br = bl.transpose(2, 0, 1).reshape(128, NPAIR)
        cr = np.ascontiguousarray(
            np.concatenate([br, g2, e2, a2], axis=1)).astype(np.float32)
        in_maps.append({"xr": xr, "wr": wr, "cr": cr})
    return in_maps


def kernel(x, weight, bias, gamma, beta, alpha, trace=False):
    x = np.asarray(x, np.float32)
    weight = np.asarray(weight, np.float32)
    bias = np.asarray(bias, np.float32)
    gamma = np.asarray(gamma, np.float32)
    beta = np.asarray(beta, np.float32)
    alpha = np.asarray(alpha, np.float32)

    if "nc" not in _cache:
        _cache["nc"] = _build()
    nc = _cache["nc"]
    in_maps = _prep(x, weight, bias, gamma, beta, alpha)
    res = run_bass_kernel_spmd(nc, in_maps, list(range(NCORES)), trace=trace)
    kernel._last = res

    out = np.empty((B, OUT_C, L, T), np.float32)
    for r in range(NCORES):
        yo = res.results[r]["yo"].astype(np.float32)
        l0 = r * L_LOC
        # yo[pi*64+c, pair, ((bh, nt, bi), t)] -> out[4bh+2nt+bi, c, l0+2pair+pi, t]
        yr = yo.reshape(2, OUT_C, NPAIR, 2, 2, 2, T)
        out[:, :, l0:l0 + L_LOC, :] = yr.transpose(3, 4, 5, 1, 2, 0, 6).reshape(
            B, OUT_C, L_LOC, T)
    return out


# revision 3
# speedup vs baseline: 1.2245x; 1.2245x over previous
"""Trainium2 Bass kernel for LocalDenseConv2D + BatchNorm + PReLU (v7: v6 +
  - x chunks split across both HW DGE queues so the x stream completes
    before the runtime CC barrier begins throttling DMA (~20us in),
  - last-pair drain restructured: Square-from-PSUM (S2) and a vector
    reduce-from-PSUM + 2048*bias fold (S) run in parallel; the ot drain
    Identity runs after, off the collective trigger path).

Sharding: out_l across 8 cores (8 rows each), all batches per core.

v3 changes vs v2:
  - x DRAM/SBUF layout [128, SLAB, 4, TP]: slab-row DMAs are contiguous
    2064B per partition (4KB-class packets, ~2x DMA bandwidth).
  - DMA split across the two HWDGE queues: x rows on sync (SP), weights +
    consts + half the gathers/stores on scalar (Activation).
  - Early dummy Sqrt activation loads the sqrt table set during the DMA
    preamble; Identity/Prelu are fillers in every set, so no table load
    sits on the post-collective critical path.
  - BN stats via sums: the PSUM-drain Identity produces sum((conv+bias))
    through accum_out; a vector tensor_tensor_reduce on the bf16 ot tile
    produces sum((conv+bias)^2). AllGather ships (S, S2) per (parity,
    channel); a single tensor_reduce merges the 16 groups. This also
    fixes v1/v2's stats-missing-bias bug.
  - ot is bf16 (halves SBUF + doubles DVE throughput on the apply).
  - Apply split: pairs 0-1 scalar Prelu, pairs 2-3 DVE 3-op prelu
    (max(t, alpha*t), valid for alpha in [0,1]).
"""
import sys
import numpy as np
import ml_dtypes

if '/opt/trn_rl_repo' not in sys.path:
    sys.path.insert(0, '/opt/trn_rl_repo')

import concourse.bass as bass
import concourse.bacc as bacc
import concourse.mybir as mybir
import concourse.tile as tile
from concourse.bass_utils import run_bass_kernel_spmd

F32 = mybir.dt.float32
BF16 = mybir.dt.bfloat16
AF = mybir.ActivationFunctionType
ALU = mybir.AluOpType
BFNP = ml_dtypes.bfloat16

B, IN_C, L, T = 8, 64, 64, 256
OUT_C, OUT_L = 64, 64
NCORES = 8
L_LOC = L // NCORES          # 8 out_l rows per core
NPAIR = L_LOC // 2           # 4 lp pairs (even/odd parity on PSUM halves)
SLAB = L_LOC + 2             # 10 x-rows incl. halo
TP = T + 2                   # padded t
EPS = 1e-5
N_GLOBAL = B * L * T         # 131072

_cache = {}


def _build():
    nc = bacc.Bacc("TRN2", target_bir_lowering=False, debug=False,
                   num_devices=NCORES)
    xr = nc.dram_tensor("xr", [128, SLAB, 4, TP], BF16, kind="ExternalInput")
    wr = nc.dram_tensor("wr", [128, 9, L_LOC, OUT_C], BF16, kind="ExternalInput")
    # combined consts: [0:NPAIR]=bias pairs, NPAIR=gamma, NPAIR+1=beta, NPAIR+2=alpha
    cr = nc.dram_tensor("cr", [128, NPAIR + 3], F32, kind="ExternalInput")
    yo = nc.dram_tensor("yo", [128, NPAIR, 2048], BF16, kind="ExternalOutput")

    cc_in = nc.dram_tensor("cc_in", [128, 2], F32)
    cc_out = nc.dram_tensor("cc_out", [NCORES * 128, 2], F32, addr_space="Shared")
    cc_din = nc.dram_tensor("cc_din", [1, 1], mybir.dt.uint8)
    cc_dout = nc.dram_tensor("cc_dout", [NCORES, 1], mybir.dt.uint8, addr_space="Shared")

    with tile.TileContext(nc) as tc:
        with (
            tc.tile_pool(name="const", bufs=1) as cpool,
            tc.tile_pool(name="xp", bufs=1) as xpool,
            tc.tile_pool(name="op", bufs=1) as opool,
            tc.tile_pool(name="fp", bufs=3) as fpool,
            tc.tile_pool(name="ps", bufs=2, space="PSUM") as ppool,
        ):
            wt = cpool.tile([128, 9, L_LOC, OUT_C], BF16)
            ct = cpool.tile([128, NPAIR + 3], F32)
            xt = xpool.tile([128, SLAB, 4, TP], BF16)
            ot = opool.tile([128, NPAIR, 2048], BF16)
            acst = cpool.tile([128, 2, NPAIR], F32)
            epst = cpool.tile([128, 1], F32)
            dum = cpool.tile([128, 1], F32)

            # weights + consts on the scalar HWDGE queue, x on sync
            nc.scalar.dma_start(wt[:, 0:1], wr.ap()[:, 0:1])
            for s in range(0, SLAB, 2):
                if (s // 2) % 2 == 0:
                    nc.sync.dma_start(xt[:, s:s + 2], xr.ap()[:, s:s + 2])
                else:
                    nc.scalar.dma_start(xt[:, s:s + 2], xr.ap()[:, s:s + 2])
            nc.scalar.dma_start(ct[:], cr.ap())
            nc.scalar.dma_start(wt[:, 1:9], wr.ap()[:, 1:9])
            # preload the sqrt table set while DMAs stream
            nc.vector.memset(epst[:], EPS)
            nc.scalar.activation(dum[:], epst[:], AF.Sqrt, bias=epst[:])
            # dummy collective at launch: pre-pay CC stream setup
            dm2 = cpool.tile([1, 1], mybir.dt.uint8)
            nc.vector.memset(dm2[:], 1)
            nc.scalar.dma_start(cc_din.ap(), dm2[:])
            nc.gpsimd.collective_compute(
                "AllGather", ALU.bypass,
                replica_groups=[list(range(NCORES))],
                ins=[cc_din[:]], outs=[cc_dout[:]])

            sb3 = cpool.tile([128, 1], F32)
            rsum = cpool.tile([128, 1], F32)
            nc.vector.tensor_scalar(sb3[:], ct[:, NPAIR - 1:NPAIR], 2048.0, None,
                                    ALU.mult)

            # ---- conv: 4 concurrent 64x64 PE tiles (bh rows x parity cols)
            for pair in range(NPAIR):
                pt = ppool.tile([128, 2, 2, 512], F32, tag="p")
                for combo in range(9):
                    di, dj = combo // 3, combo % 3
                    first = combo == 0
                    last = combo == 8
                    for nt in range(2):
                        for bh in range(2):
                            for pi in range(2):
                                lp = 2 * pair + pi
                                nc.tensor.matmul(
                                    pt[64 * pi:64 * pi + 64, bh, nt, :],
                                    wt[64 * bh:64 * bh + 64, combo, lp, :],
                                    xt[64 * bh:64 * bh + 64,
                                       lp + di,
                                       2 * nt:2 * nt + 2,
                                       dj:dj + T],
                                    start=first, stop=last)
                sq = fpool.tile([128, 2048], BF16, tag="sq")
                if pair < NPAIR - 1:
                    # drain + sum((conv+bias)) via accum, then sumsq
                    nc.scalar.activation(
                        ot[:, pair, :],
                        pt[:].rearrange("p a b n -> p (a b n)"),
                        AF.Identity, bias=ct[:, pair:pair + 1],
                        accum_out=acst[:, 0, pair:pair + 1])
                    nc.scalar.activation(
                        sq[:], ot[:, pair, :], AF.Square,
                        accum_out=acst[:, 1, pair:pair + 1])
                else:
                    # last pair: trigger path needs only S and S2.
                    # scalar: S2 = sum((psum+bias)^2); vector: S via
                    # reduce(psum) + 2048*bias; Identity drain afterwards.
                    nc.scalar.activation(
                        sq[:], pt[:].rearrange("p a b n -> p (a b n)"),
                        AF.Square, bias=ct[:, pair:pair + 1],
                        accum_out=acst[:, 1, pair:pair + 1])
                    nc.vector.tensor_reduce(
                        rsum[:], pt[:].rearrange("p a b n -> p (a b n)"),
                        mybir.AxisListType.X, ALU.add)
                    nc.vector.tensor_tensor(
                        acst[:, 0, pair:pair + 1], rsum[:], sb3[:], ALU.add)
                    nc.scalar.activation(
                        ot[:, pair, :],
                        pt[:].rearrange("p a b n -> p (a b n)"),
                        AF.Identity, bias=ct[:, pair:pair + 1])

            # ---- local sums -> collective -> global scale/shift ----
            loc = cpool.tile([128, 2, 1], F32)
            nc.vector.tensor_reduce(loc[:], acst[:], mybir.AxisListType.X, ALU.add)
            nc.sync.dma_start(cc_in.ap(), loc[:, :, 0])
            nc.gpsimd.collective_compute(
                "AllGather", ALU.bypass,
                replica_groups=[list(range(NCORES))],
                ins=[cc_in[:]], outs=[cc_out[:]])
            gath = cpool.tile([128, 2, 2 * NCORES], F32)
            nc.sync.dma_start(
                gath[0:64],
                cc_out.ap().rearrange("(r h c) s -> c s (r h)", h=2, c=OUT_C))
            nc.scalar.dma_start(
                gath[64:128],
                cc_out.ap().rearrange("(r h c) s -> c s (r h)", h=2, c=OUT_C))

            gstat = cpool.tile([128, 2, 1], F32)
            nc.vector.tensor_reduce(gstat[:], gath[:], mybir.AxisListType.X, ALU.add)
            gsc = cpool.tile([128, 2], F32)
            nc.vector.tensor_scalar(gsc[:], gstat[:, :, 0], 1.0 / N_GLOBAL, None,
                                    ALU.mult)
            # var = E[x^2] - mean^2 ; sca = gamma/sqrt(var+eps)
            # shi = beta - mean*sca
            var = cpool.tile([128, 1], F32)
            std = cpool.tile([128, 1], F32)
            rstd = cpool.tile([128, 1], F32)
            sca = cpool.tile([128, 1], F32)
            shi = cpool.tile([128, 1], F32)
            nc.vector.tensor_tensor(var[:], gsc[:, 0:1], gsc[:, 0:1], ALU.mult)
            nc.vector.tensor_tensor(var[:], gsc[:, 1:2], var[:], ALU.subtract)
            nc.scalar.activation(std[:], var[:], AF.Sqrt, bias=epst[:])
            nc.vector.reciprocal(rstd[:], std[:])
            nc.vector.tensor_tensor(sca[:], ct[:, NPAIR:NPAIR + 1], rstd[:], ALU.mult)
            nc.vector.tensor_tensor(shi[:], gsc[:, 0:1], sca[:], ALU.mult)
            nc.vector.tensor_tensor(shi[:], ct[:, NPAIR + 1:NPAIR + 2], shi[:], ALU.subtract)

            # ---- fused BN-apply + PReLU + store (bf16) ----
            # pairs 0-1 on scalar (Prelu), pairs 2-3 on DVE (max(t, alpha*t))
            for pair in range(2):
                fo = fpool.tile([128, 2048], BF16, tag="fo")
                nc.scalar.activation(
                    fo[:], ot[:, pair, :],
                    AF.Prelu, bias=shi[:], scale=sca[:], alpha=ct[:, NPAIR + 2:NPAIR + 3])
                nc.scalar.dma_start(yo.ap()[:, pair, :], fo[:])
            for pair in range(2, NPAIR):
                tv = fpool.tile([128, 2048], BF16, tag="tv")
                tu = fpool.tile([128, 2048], BF16, tag="tu")
                nc.vector.tensor_scalar(tv[:], ot[:, pair, :], sca[:], shi[:],
                                        ALU.mult, ALU.add)
                nc.vector.tensor_scalar(tu[:], tv[:], ct[:, NPAIR + 2:NPAIR + 3], None, ALU.mult)
                nc.vector.tensor_tensor(tu[:], tv[:], tu[:], ALU.max)
                nc.sync.dma_start(yo.ap()[:, pair, :], tu[:])
    nc.compile()
    return nc


def _prep(x, weight, bias, gamma, beta, alpha):
    """Build per-core input maps (host-side shard + relayout, bf16)."""
    in_maps = []
    xpad = np.zeros((B, IN_C, L + 2, TP), np.float32)
    xpad[:, :, 1:L + 1, 1:T + 1] = x
    xpad = xpad.astype(BFNP)
    wbf = weight.astype(BFNP)
    g2 = np.concatenate([gamma, gamma]).reshape(128, 1).astype(np.float32)
    e2 = np.concatenate([beta, beta]).reshape(128, 1).astype(np.float32)
    a2 = np.full((128, 1), float(alpha[0]), np.float32)
    for r in range(NCORES):
        l0 = r * L_LOC
        # x: [bh*64+ci, slab_row, b4, 1+t]
        slab = xpad[:, :, l0:l0 + SLAB, :]          # (B, IN_C, SLAB, TP)
        xr = slab.reshape(2, 4, IN_C, SLAB, TP).transpose(0, 2, 3, 1, 4)
        xr = np.ascontiguousarray(xr.reshape(128, SLAB, 4, TP))
        # weight: [bh*64+ci, combo, lp, c] = weight[ci*9+combo, c, l0+lp]
        wl = wbf[:, :, l0:l0 + L_LOC]               # (576, 64, 8)
        wl = wl.reshape(IN_C, 9, OUT_C, L_LOC).transpose(0, 1, 3, 2)
        wr = np.ascontiguousarray(
            np.broadcast_to(wl[None], (2, IN_C, 9, L_LOC, OUT_C))
            .reshape(128, 9, L_LOC, OUT_C))
        # bias: [pi*64+c, pair] = bias[c, l0 + 2*pair + pi]
        bl = bias[:, l0:l0 + L_LOC].reshape(OUT_C, NPAIR, 2)
        br = bl.transpose(2, 0, 1).reshape(128, NPAIR)
        cr = np.ascontiguousarray(
            np.concatenate([br, g2, e2, a2], axis=1)).astype(np.float32)
        in_maps.append({"xr": xr, "wr": wr, "cr": cr})
    return in_maps


def kernel(x, weight, bias, gamma, beta, alpha, trace=False):
    x = np.asarray(x, np.float32)
    weight = np.asarray(weight, np.float32)
    bias = np.asarray(bias, np.float32)
    gamma = np.asarray(gamma, np.float32)
    beta = np.asarray(beta, np.float32)
    alpha = np.asarray(alpha, np.float32)

    if "nc" not in _cache:
        _cache["nc"] = _build()
    nc = _cache["nc"]
    in_maps = _prep(x, weight, bias, gamma, beta, alpha)
    res = run_bass_kernel_spmd(nc, in_maps, list(range(NCORES)), trace=trace)
    kernel._last = res

    out = np.empty((B, OUT_C, L, T), np.float32)
    for r in range(NCORES):
        yo = res.results[r]["yo"].astype(np.float32)
        l0 = r * L_LOC
        # yo[pi*64+c, pair, ((bh, nt, bi), t)] -> out[4bh+2nt+bi, c, l0+2pair+pi, t]
        yr = yo.reshape(2, OUT_C, NPAIR, 2, 2, 2, T)
        out[:, :, l0:l0 + L_LOC, :] = yr.transpose(3, 4, 5, 1, 2, 0, 6).reshape(
            B, OUT_C, L_LOC, T)
    return out
